# revision 11
# baseline (speedup 1.0000x reference)
"""DaGMM GCN-encoder kernel for 8 Trainium2 NeuronCores.

Model (reference):
    h1  = relu(adj @ (x @ w1) + b1)        # [N, 128]
    h2  = relu(adj @ (h1 @ w2) + b2)       # [N, 32]
    enc = adj @ (h2 @ w3) + b3             # [N, 4]
    out = segment_mean(enc, boundaries)    # [G, 4]
    gamma = softmax(relu(out @ ew1 + eb1) @ ew2 + eb2)  # [G, 10]
    returns (x, out, gamma)

Sharding: row-parallel over the adjacency matrix. Core i owns output rows
R_i = [2048*i, 2048*(i+1)) and streams adj[R_i, :].T (fed pre-transposed
from host so the contraction dim lands on SBUF partitions), once per GCN
layer. Activations are kept transposed ([feat, node]) on device so no
fp32 transposes are ever needed:

  MM-A: Z1[c,f]   = sum_d xT[d,c] * w1[d,f]          (lhsT=xT tile, rhs=w1)
  MM-B: h1T[f,r]  = relu(sum_c Z1[c,f] * adjT[c,r] + b1)
  MM-C: Z2[r,g]   = sum_f h1T[f,r] * w2[f,g]         -> AllGather -> Z2[c,g]
  MM-D: h2T[g,r]  = relu(sum_c Z2[c,g] * adjT[c,r] + b2)
  MM-E: Z3[r,l]   = sum_g h2T[g,r] * w3[g,l]         -> AllGather -> Z3[c,l]

Two variants, built lazily depending on the segment layout of the actual
input boundaries:

* "pool" (boundaries are the reference's equal 32-node segments): the
  third adj pass is folded away.  While passes B/D stream adj tiles, the
  (otherwise idle) VectorEngine computes per-segment row sums
  PT[c, g] = sum_{r in seg g} adjT[c, r]; then
  out[g] = (sum_c Z3[c,:] * PT[c,g]) / 32 + b3 is a tiny matmul.  This
  removes 1/3 of the HBM traffic.

* "general" (any sorted boundaries): a third adj pass computes
  encT[l,r] = sum_c Z3[c,l] * adjT[c,r] + b3, and the exact segment mean
  runs on host.

Matmul operands are fp16 by default (PSUM accumulation stays f32): the
TensorEngine streams 16-bit operands at 4x the fp32 rate and the
HBM-resident adj stream halves.  adj is pre-scaled by 2^14 on host so its
tiny uniform(0, 1/N) values sit in the fp16 normal range (fp16's 10-bit
mantissa then gives ~40x better accuracy than bf16); the inverse scale is
folded into the psum-eviction activations.  DAGMM_DTYPE=bf16|f32 fall
back to unscaled bf16/fp32 operands.

The tiny tail (segment mean of a [16K, 4] tensor, a [512, 4] MLP and a
softmax) runs on host in numpy with exact jax segment semantics.
"""

import os

import ml_dtypes
import numpy as np

import concourse.bass as bass  # noqa: F401  (import registers arch bits)
import concourse.mybir as mybir
import concourse.tile as tile
from concourse import bacc
from concourse.bass_utils import run_bass_kernel_spmd

N_CORES = 8
N = 16384          # nodes
G = 512            # graphs
D = 512            # input feature dim
H1 = 128
H2 = 32
L = 4              # latent
P = 128            # partitions
RL = N // N_CORES  # 2048 local output rows per core
NK = N // P        # 128 contraction subtiles over nodes
ND = D // P        # 4 contraction subtiles over input dim
NM = RL // P       # 16 local row chunks
RC = 512           # moving free-dim per matmul
NRC = RL // RC     # 4
SEG = N // G       # 32 nodes per segment in the clean case
GL = RL // SEG     # 64 local graphs per core in the clean case

F32 = mybir.dt.float32
AF = mybir.ActivationFunctionType

COMPUTE = os.environ.get("DAGMM_DTYPE", "fp16")
# adj is pre-scaled by 2^14 on host in fp16 mode so its tiny values sit in
# the fp16 normal range; the inverse folds into the psum-eviction scale.
ADJ_SCALE = 16384.0 if COMPUTE == "fp16" else 1.0
INV_SCALE = 1.0 / ADJ_SCALE

_BUILD_CACHE = {}
LAST_RESULT = None  # BassKernelResults of the most recent run (for profiling)


def _cdt():
    if COMPUTE == "f32":
        return F32
    if COMPUTE == "fp16":
        return mybir.dt.float16
    return mybir.dt.bfloat16


def _cnp():
    if COMPUTE == "f32":
        return np.float32
    if COMPUTE == "fp16":
        return np.float16
    return ml_dtypes.bfloat16


def _build(variant):
    key = (variant, COMPUTE)
    if key in _BUILD_CACHE:
        return _BUILD_CACHE[key]
    CDT = _cdt()
    pool_mode = variant == "pool"
    nc = bacc.Bacc(None, target_bir_lowering=False, num_devices=N_CORES)

    adjt = nc.dram_tensor("adjt", [N, RL], CDT, kind="ExternalInput")
    xt = nc.dram_tensor("xt", [D, N], CDT, kind="ExternalInput")
    w1 = nc.dram_tensor("w1", [D, H1], CDT, kind="ExternalInput")
    b1 = nc.dram_tensor("b1", [H1, 1], F32, kind="ExternalInput")
    w2 = nc.dram_tensor("w2", [H1, H2], CDT, kind="ExternalInput")
    b2 = nc.dram_tensor("b2", [H2, 1], F32, kind="ExternalInput")
    w3 = nc.dram_tensor("w3", [H2, L], CDT, kind="ExternalInput")
    b3 = nc.dram_tensor("b3", [L, 1], F32, kind="ExternalInput")
    if pool_mode:
        poolt = nc.dram_tensor("poolt", [L, GL], F32, kind="ExternalOutput")
    else:
        enct = nc.dram_tensor("enct", [L, RL], F32, kind="ExternalOutput")

    with tile.TileContext(nc) as tc:
        with (
            tc.tile_pool(name="const", bufs=1) as const,
            tc.tile_pool(name="zbig", bufs=1) as zbig,
            tc.tile_pool(name="acts", bufs=1) as acts,
            tc.tile_pool(name="xtp", bufs=5) as xtp,
            tc.tile_pool(name="stream", bufs=20) as stream,
            tc.tile_pool(name="evict", bufs=4) as evict,
            tc.tile_pool(name="ps1", bufs=4, space="PSUM") as ps1,
            tc.tile_pool(name="psacc", bufs=4, space="PSUM") as psacc,
            tc.tile_pool(name="dram", bufs=1, space="DRAM") as dram,
        ):
            # ---- constants ----
            w1t = const.tile([P, ND, H1], CDT)
            nc.sync.dma_start(w1t[:], w1[:].rearrange("(o p) f -> p o f", p=P))
            w2t = const.tile([H1, H2], CDT)
            nc.sync.dma_start(w2t[:], w2[:])
            w3t = const.tile([H2, L], CDT)
            nc.sync.dma_start(w3t[:], w3[:])
            b1t = const.tile([H1, 1], F32)
            nc.sync.dma_start(b1t[:], b1[:])
            b2t = const.tile([H2, 1], F32)
            nc.sync.dma_start(b2t[:], b2[:])
            b3t = const.tile([L, 1], F32)
            nc.sync.dma_start(b3t[:], b3[:])

            # tiny warmup AllGather, dependency-free: absorbs the one-time
            # collective-path setup so the real z2 gather runs at steady cost
            warm_in = dram.tile([P, 1], F32)
            warm_out = dram.tile([N_CORES, P, 1], F32, addr_space="Shared")
            nc.gpsimd.collective_compute(
                "AllGather", mybir.AluOpType.bypass,
                replica_groups=[list(range(N_CORES))],
                ins=[warm_in[:].opt()], outs=[warm_out[:].opt()],
            )

            Z1 = zbig.tile([P, NK, H1], CDT)       # (x @ w1), node-major
            h1T = acts.tile([H1, RL], CDT)
            h2T = acts.tile([H2, RL], CDT)
            if pool_mode:
                PT = zbig.tile([P, NK, GL], F32)   # per-segment adj row sums
            else:
                encT = acts.tile([L, RL], F32)

            # ---- phase A: Z1[c, f] = x @ w1 (full, redundant per core) ----
            xt_r = xt[:].rearrange("(o p) c -> p o c", p=P)
            XC = 512  # c-columns per xt DMA
            for cc in range(NK):
                if cc % (XC // P) == 0:
                    xtt = xtp.tile([P, ND, XC], CDT)
                    nc.sync.dma_start(
                        xtt[:], xt_r[:, :, cc * P:cc * P + XC])
                ci = (cc % (XC // P)) * P
                ps = ps1.tile([P, H1], F32)
                for ds in range(ND):
                    nc.tensor.matmul(
                        ps[:], lhsT=xtt[:, ds, ci:ci + P], rhs=w1t[:, ds, :],
                        start=(ds == 0), stop=(ds == ND - 1),
                    )
                nc.vector.tensor_copy(Z1[:, cc, :], ps[:])

            # ---- phase B: h1T = relu(Z1.T @ adjT + b1) ----
            psB = [psacc.tile([P, RC], F32, name=f"psB{i}", tag="accbank")
                   for i in range(NRC)]
            for k in range(NK):
                at = stream.tile([P, RL], CDT, tag="adjstream")
                nc.sync.dma_start(at[:], adjt[k * P:(k + 1) * P, :])
                if pool_mode and k < NK // 2:
                    nc.vector.reduce_sum(
                        PT[:, k, :],
                        at[:].rearrange("p (g s) -> p g s", s=SEG),
                        axis=mybir.AxisListType.X,
                    )
                for rc in range(NRC):
                    nc.tensor.matmul(
                        psB[rc][:], lhsT=Z1[:, k, :],
                        rhs=at[:, rc * RC:(rc + 1) * RC],
                        start=(k == 0), stop=(k == NK - 1),
                    )
            for rc in range(NRC):
                sl = slice(rc * RC, (rc + 1) * RC)
                nc.vector.tensor_scalar(
                    h1T[:, sl], psB[rc][:], INV_SCALE, b1t[:],
                    mybir.AluOpType.mult, mybir.AluOpType.add,
                )
                nc.vector.tensor_scalar(
                    h1T[:, sl], h1T[:, sl], 0.0, None, mybir.AluOpType.max,
                )

            # ---- phase C: Z2 local rows + AllGather ----
            # swizzled exchange layout: each rank ships [P, NM, H2] blocks so
            # the gathered reload is 8 block-DMAs with 1 KiB descriptor runs
            z2loc = dram.tile([P, NM, H2], CDT)
            z2full = dram.tile([N_CORES, P, NM, H2], CDT, addr_space="Shared")
            for mc in range(NM):
                ps = ps1.tile([P, H2], F32)
                nc.tensor.matmul(ps[:], lhsT=h1T[:, mc * P:(mc + 1) * P],
                                 rhs=w2t[:])
                ev = evict.tile([P, H2], CDT, tag="ev2")
                nc.vector.tensor_copy(ev[:], ps[:])
                nc.sync.dma_start(z2loc[:, mc, :], ev[:])
            nc.gpsimd.collective_compute(
                "AllGather", mybir.AluOpType.bypass,
                replica_groups=[list(range(N_CORES))],
                ins=[z2loc[:].opt()], outs=[z2full[:].opt()],
            )
            # gpsimd queue: keeps the sync queue free to prefetch the next
            # pass's adj stream while the collective completes
            Z2 = acts.tile([P, NK, H2], CDT)
            for j in range(N_CORES):
                nc.gpsimd.dma_start(Z2[:, j * NM:(j + 1) * NM, :],
                                    z2full[j, :, :, :])

            # ---- phase D: h2T = relu(Z2.T @ adjT + b2) ----
            psD = [psacc.tile([H2, RC], F32, name=f"psD{i}", tag="accbank")
                   for i in range(NRC)]
            for k in range(NK):
                at = stream.tile([P, RL], CDT, tag="adjstream")
                nc.sync.dma_start(at[:], adjt[k * P:(k + 1) * P, :])
                if pool_mode and k >= NK // 2:
                    nc.vector.reduce_sum(
                        PT[:, k, :],
                        at[:].rearrange("p (g s) -> p g s", s=SEG),
                        axis=mybir.AxisListType.X,
                    )
                for rc in range(NRC):
                    nc.tensor.matmul(
                        psD[rc][:], lhsT=Z2[:, k, :],
                        rhs=at[:, rc * RC:(rc + 1) * RC],
                        start=(k == 0), stop=(k == NK - 1),
                    )
            for rc in range(NRC):
                sl = slice(rc * RC, (rc + 1) * RC)
                nc.vector.tensor_scalar(
                    h2T[:, sl], psD[rc][:], INV_SCALE, b2t[:],
                    mybir.AluOpType.mult, mybir.AluOpType.add,
                )
                nc.vector.tensor_scalar(
                    h2T[:, sl], h2T[:, sl], 0.0, None, mybir.AluOpType.max,
                )

            # ---- phase E: Z3 local rows + AllGather ----
            ZDT = F32 if pool_mode else CDT  # pool matmul runs f32 (PT is f32)
            z3loc = dram.tile([P, NM, L], ZDT)
            z3full = dram.tile([N_CORES, P, NM, L], ZDT, addr_space="Shared")
            for mc in range(NM):
                ps = ps1.tile([P, L], F32)
                nc.tensor.matmul(ps[:], lhsT=h2T[:, mc * P:(mc + 1) * P],
                                 rhs=w3t[:])
                ev = evict.tile([P, L], ZDT, tag="ev3")
                nc.vector.tensor_copy(ev[:], ps[:])
                nc.sync.dma_start(z3loc[:, mc, :], ev[:])
            nc.gpsimd.collective_compute(
                "AllGather", mybir.AluOpType.bypass,
                replica_groups=[list(range(N_CORES))],
                ins=[z3loc[:].opt()], outs=[z3full[:].opt()],
            )
            Z3 = acts.tile([P, NK, L], ZDT)
            # in pool mode the adj stream is done, so the sync HWDGE queue is
            # idle and faster for this load than gpsimd SWDGE
            z3_dma = nc.sync.dma_start if pool_mode else nc.gpsimd.dma_start
            for j in range(N_CORES):
                z3_dma(Z3[:, j * NM:(j + 1) * NM, :], z3full[j, :, :, :])

            if pool_mode:
                # ---- pooled output: out[g] = (Z3.T @ PT)[:, g] / SEG + b3 ----
                psP = psacc.tile([L, GL], F32, tag="accbank")
                for k in range(NK):
                    nc.tensor.matmul(
                        psP[:], lhsT=Z3[:, k, :], rhs=PT[:, k, :],
                        start=(k == 0), stop=(k == NK - 1),
                    )
                poolT = acts.tile([L, GL], F32)
                nc.scalar.activation(
                    poolT[:], psP[:], AF.Identity,
                    bias=b3t[:], scale=INV_SCALE / SEG,
                )
                nc.sync.dma_start(poolt[:], poolT[:])
            else:
                # ---- phase F: encT = Z3.T @ adjT + b3 ----
                psF = [psacc.tile([L, RC], F32, name=f"psF{i}", tag="accbank")
                       for i in range(NRC)]
                for k in range(NK):
                    at = stream.tile([P, RL], CDT, tag="adjstream")
                    nc.sync.dma_start(at[:], adjt[k * P:(k + 1) * P, :])
                    for rc in range(NRC):
                        nc.tensor.matmul(
                            psF[rc][:], lhsT=Z3[:, k, :],
                            rhs=at[:, rc * RC:(rc + 1) * RC],
                            start=(k == 0), stop=(k == NK - 1),
                        )
                for rc in range(NRC):
                    nc.scalar.activation(
                        encT[:, rc * RC:(rc + 1) * RC], psF[rc][:],
                        AF.Identity, bias=b3t[:], scale=INV_SCALE,
                    )
                nc.sync.dma_start(enct[:], encT[:])

    nc.compile()
    _BUILD_CACHE[key] = nc
    return nc


def _np32(a):
    a = np.asarray(a)
    if a.dtype == np.float32:
        return np.ascontiguousarray(a)
    return np.ascontiguousarray(a, dtype=np.float32)


def kernel(x, adj, graph_to_last_batch, w1, b1, w2, b2, w3, b3,
           ew1, eb1, ew2, eb2):
    global LAST_RESULT
    x = _np32(x)
    adj = _np32(adj)
    boundaries = np.asarray(graph_to_last_batch)
    w1 = _np32(w1); b1 = _np32(b1)
    w2 = _np32(w2); b2 = _np32(b2)
    w3 = _np32(w3); b3 = _np32(b3)

    clean = np.array_equal(
        boundaries.astype(np.int64),
        np.arange(1, G + 1, dtype=np.int64) * SEG)
    variant = "pool" if clean else "general"
    nc = _build(variant)
    cnp = _cnp()

    xt = np.ascontiguousarray(x.T).astype(cnp)
    shared = {
        "xt": xt,
        "w1": w1.astype(cnp), "b1": b1.reshape(H1, 1),
        "w2": w2.astype(cnp), "b2": b2.reshape(H2, 1),
        "w3": w3.astype(cnp), "b3": b3.reshape(L, 1),
    }
    in_maps = []
    for i in range(N_CORES):
        adjt_i = np.ascontiguousarray(adj[i * RL:(i + 1) * RL, :].T)
        if ADJ_SCALE != 1.0:
            adjt_i = adjt_i * np.float32(ADJ_SCALE)
        adjt_i = adjt_i.astype(cnp)
        in_maps.append({"adjt": adjt_i, **shared})

    res = run_bass_kernel_spmd(nc, in_maps, core_ids=list(range(N_CORES)))
    LAST_RESULT = res

    if clean:
        out = np.concatenate(
            [res.results[i]["poolt"].T for i in range(N_CORES)],
            axis=0).astype(np.float32)  # [G, L]
    else:
        encT = np.concatenate(
            [res.results[i]["enct"] for i in range(N_CORES)], axis=1)
        enc = encT.T.astype(np.float32)  # [N, L]
        # exact jax segment-mean semantics
        seg = np.searchsorted(boundaries, np.arange(N), side="right")
        valid = seg < G
        sums = np.zeros((G, L), np.float32)
        np.add.at(sums, seg[valid], enc[valid])
        cnts = np.zeros((G,), np.float32)
        np.add.at(cnts, seg[valid], np.float32(1.0))
        with np.errstate(divide="ignore", invalid="ignore"):
            out = (sums / cnts[:, None]).astype(np.float32)

    ew1 = _np32(ew1); eb1 = _np32(eb1)
    ew2 = _np32(ew2); eb2 = _np32(eb2)
    e = np.maximum(out @ ew1 + eb1, 0.0).astype(np.float32)
    logits = (e @ ew2 + eb2).astype(np.float32)
    zm = logits - logits.max(axis=1, keepdims=True)
    ez = np.exp(zm)
    gamma = (ez / ez.sum(axis=1, keepdims=True)).astype(np.float32)

    return (x, out, gamma)


# revision 12
# speedup vs baseline: 1.0566x; 1.0566x over previous
"""DaGMM GCN-encoder kernel for 8 Trainium2 NeuronCores.

Model (reference):
    h1  = relu(adj @ (x @ w1) + b1)        # [N, 128]
    h2  = relu(adj @ (h1 @ w2) + b2)       # [N, 32]
    enc = adj @ (h2 @ w3) + b3             # [N, 4]
    out = segment_mean(enc, boundaries)    # [G, 4]
    gamma = softmax(relu(out @ ew1 + eb1) @ ew2 + eb2)  # [G, 10]
    returns (x, out, gamma)

Sharding: row-parallel over the adjacency matrix. Core i owns output rows
R_i = [2048*i, 2048*(i+1)) and streams adj[R_i, :].T (fed pre-transposed
from host so the contraction dim lands on SBUF partitions), once per GCN
layer. Activations are kept transposed ([feat, node]) on device so no
fp32 transposes are ever needed:

  MM-A: Z1[c,f]   = sum_d xT[d,c] * w1[d,f]          (lhsT=xT tile, rhs=w1)
  MM-B: h1T[f,r]  = relu(sum_c Z1[c,f] * adjT[c,r] + b1)
  MM-C: Z2[r,g]   = sum_f h1T[f,r] * w2[f,g]         -> AllGather -> Z2[c,g]
  MM-D: h2T[g,r]  = relu(sum_c Z2[c,g] * adjT[c,r] + b2)
  MM-E: Z3[r,l]   = sum_g h2T[g,r] * w3[g,l]         -> AllGather -> Z3[c,l]

Two variants, built lazily depending on the segment layout of the actual
input boundaries:

* "pool" (boundaries are the reference's equal 32-node segments): the
  third adj pass is folded away.  While passes B/D stream adj tiles, the
  (otherwise idle) VectorEngine computes per-segment row sums
  PT[c, g] = sum_{r in seg g} adjT[c, r]; then
  out[g] = (sum_c Z3[c,:] * PT[c,g]) / 32 + b3 is a tiny matmul.  This
  removes 1/3 of the HBM traffic.

* "general" (any sorted boundaries): a third adj pass computes
  encT[l,r] = sum_c Z3[c,l] * adjT[c,r] + b3, and the exact segment mean
  runs on host.

Matmul operands are fp16 by default (PSUM accumulation stays f32): the
TensorEngine streams 16-bit operands at 4x the fp32 rate and the
HBM-resident adj stream halves.  adj is pre-scaled by 2^14 on host so its
tiny uniform(0, 1/N) values sit in the fp16 normal range (fp16's 10-bit
mantissa then gives ~40x better accuracy than bf16); the inverse scale is
folded into the psum-eviction activations.  DAGMM_DTYPE=bf16|f32 fall
back to unscaled bf16/fp32 operands.

The tiny tail (segment mean of a [16K, 4] tensor, a [512, 4] MLP and a
softmax) runs on host in numpy with exact jax segment semantics.
"""

import os

import ml_dtypes
import numpy as np

import concourse.bass as bass  # noqa: F401  (import registers arch bits)
import concourse.mybir as mybir
import concourse.tile as tile
from concourse import bacc
from concourse.bass_utils import run_bass_kernel_spmd

N_CORES = 8
N = 16384          # nodes
G = 512            # graphs
D = 512            # input feature dim
H1 = 128
H2 = 32
L = 4              # latent
P = 128            # partitions
RL = N // N_CORES  # 2048 local output rows per core
NK = N // P        # 128 contraction subtiles over nodes
ND = D // P        # 4 contraction subtiles over input dim
NM = RL // P       # 16 local row chunks
RC = 512           # moving free-dim per matmul
NRC = RL // RC     # 4
SEG = N // G       # 32 nodes per segment in the clean case
GL = RL // SEG     # 64 local graphs per core in the clean case

F32 = mybir.dt.float32
AF = mybir.ActivationFunctionType

COMPUTE = os.environ.get("DAGMM_DTYPE", "fp16")
# adj is pre-scaled by 2^14 on host in fp16 mode so its tiny values sit in
# the fp16 normal range; the inverse folds into the psum-eviction scale.
ADJ_SCALE = 16384.0 if COMPUTE == "fp16" else 1.0
INV_SCALE = 1.0 / ADJ_SCALE

_BUILD_CACHE = {}
LAST_RESULT = None  # BassKernelResults of the most recent run (for profiling)


def _cdt():
    if COMPUTE == "f32":
        return F32
    if COMPUTE == "fp16":
        return mybir.dt.float16
    return mybir.dt.bfloat16


def _cnp():
    if COMPUTE == "f32":
        return np.float32
    if COMPUTE == "fp16":
        return np.float16
    return ml_dtypes.bfloat16


def _build(variant):
    key = (variant, COMPUTE)
    if key in _BUILD_CACHE:
        return _BUILD_CACHE[key]
    CDT = _cdt()
    pool_mode = variant == "pool"
    nc = bacc.Bacc(None, target_bir_lowering=False, num_devices=N_CORES)

    adjt = nc.dram_tensor("adjt", [N, RL], CDT, kind="ExternalInput")
    xt = nc.dram_tensor("xt", [D, N], CDT, kind="ExternalInput")
    w1 = nc.dram_tensor("w1", [D, H1], CDT, kind="ExternalInput")
    b1 = nc.dram_tensor("b1", [H1, 1], F32, kind="ExternalInput")
    w2 = nc.dram_tensor("w2", [H1, H2], CDT, kind="ExternalInput")
    b2 = nc.dram_tensor("b2", [H2, 1], F32, kind="ExternalInput")
    w3 = nc.dram_tensor("w3", [H2, L], CDT, kind="ExternalInput")
    b3 = nc.dram_tensor("b3", [L, 1], F32, kind="ExternalInput")
    if pool_mode:
        poolt = nc.dram_tensor("poolt", [L, GL], F32, kind="ExternalOutput")
    else:
        enct = nc.dram_tensor("enct", [L, RL], F32, kind="ExternalOutput")

    with tile.TileContext(nc) as tc:
        with (
            tc.tile_pool(name="const", bufs=1) as const,
            tc.tile_pool(name="zbig", bufs=1) as zbig,
            tc.tile_pool(name="acts", bufs=1) as acts,
            tc.tile_pool(name="xtp", bufs=5) as xtp,
            tc.tile_pool(name="stream", bufs=20) as stream,
            tc.tile_pool(name="evict", bufs=4) as evict,
            tc.tile_pool(name="ps1", bufs=4, space="PSUM") as ps1,
            tc.tile_pool(name="psacc", bufs=4, space="PSUM") as psacc,
            tc.tile_pool(name="dram", bufs=1, space="DRAM") as dram,
        ):
            # ---- constants ----
            w1t = const.tile([P, ND, H1], CDT)
            nc.sync.dma_start(w1t[:], w1[:].rearrange("(o p) f -> p o f", p=P))
            w2t = const.tile([H1, H2], CDT)
            nc.sync.dma_start(w2t[:], w2[:])
            w3t = const.tile([H2, L], CDT)
            nc.sync.dma_start(w3t[:], w3[:])
            b1t = const.tile([H1, 1], F32)
            nc.sync.dma_start(b1t[:], b1[:])
            b2t = const.tile([H2, 1], F32)
            nc.sync.dma_start(b2t[:], b2[:])
            b3t = const.tile([L, 1], F32)
            nc.sync.dma_start(b3t[:], b3[:])

            # tiny warmup AllGather, dependency-free: absorbs the one-time
            # collective-path setup so the real z2 gather runs at steady cost
            warm_in = dram.tile([P, 1], F32)
            warm_out = dram.tile([N_CORES, P, 1], F32, addr_space="Shared")
            nc.gpsimd.collective_compute(
                "AllGather", mybir.AluOpType.bypass,
                replica_groups=[list(range(N_CORES))],
                ins=[warm_in[:].opt()], outs=[warm_out[:].opt()],
            )

            Z1 = zbig.tile([P, NK, H1], CDT)       # (x @ w1), node-major
            h1T = acts.tile([H1, RL], CDT)
            h2T = acts.tile([H2, RL], CDT)
            if pool_mode:
                PT = zbig.tile([P, NK, GL], F32)   # per-segment adj row sums
            else:
                encT = acts.tile([L, RL], F32)

            # ---- phase A: Z1[c, f] = x @ w1 (full, redundant per core) ----
            xt_r = xt[:].rearrange("(o p) c -> p o c", p=P)
            XC = 512  # c-columns per xt DMA
            for cc in range(NK):
                if cc % (XC // P) == 0:
                    xtt = xtp.tile([P, ND, XC], CDT)
                    nc.sync.dma_start(
                        xtt[:], xt_r[:, :, cc * P:cc * P + XC])
                ci = (cc % (XC // P)) * P
                ps = ps1.tile([P, H1], F32)
                for ds in range(ND):
                    nc.tensor.matmul(
                        ps[:], lhsT=xtt[:, ds, ci:ci + P], rhs=w1t[:, ds, :],
                        start=(ds == 0), stop=(ds == ND - 1),
                    )
                nc.vector.tensor_copy(Z1[:, cc, :], ps[:])

            # ---- phase B: h1T = relu(Z1.T @ adjT + b1) ----
            psB = [psacc.tile([P, RC], F32, name=f"psB{i}", tag="accbank")
                   for i in range(NRC)]
            for k in range(NK):
                at = stream.tile([P, RL], CDT, tag="adjstream")
                nc.sync.dma_start(at[:], adjt[k * P:(k + 1) * P, :])
                if pool_mode and k < NK // 2:
                    nc.vector.reduce_sum(
                        PT[:, k, :],
                        at[:].rearrange("p (g s) -> p g s", s=SEG),
                        axis=mybir.AxisListType.X,
                    )
                for rc in range(NRC):
                    nc.tensor.matmul(
                        psB[rc][:], lhsT=Z1[:, k, :],
                        rhs=at[:, rc * RC:(rc + 1) * RC],
                        start=(k == 0), stop=(k == NK - 1),
                    )
            for rc in range(NRC):
                sl = slice(rc * RC, (rc + 1) * RC)
                nc.vector.tensor_scalar(
                    h1T[:, sl], psB[rc][:], INV_SCALE, b1t[:],
                    mybir.AluOpType.mult, mybir.AluOpType.add,
                )
                nc.vector.tensor_scalar(
                    h1T[:, sl], h1T[:, sl], 0.0, None, mybir.AluOpType.max,
                )

            # ---- phase C: Z2 local rows + AllGather ----
            # swizzled exchange layout: each rank ships [P, NM/2, H2] blocks;
            # the gather is split in two halves so pass D can start on the
            # first half's k-subtiles while the second half is still in
            # flight (the consuming k-loop below is reordered to match)
            NMH = NM // 2
            z2locA = dram.tile([P, NMH, H2], CDT)
            z2locB = dram.tile([P, NMH, H2], CDT)
            z2fullA = dram.tile([N_CORES, P, NMH, H2], CDT, addr_space="Shared")
            z2fullB = dram.tile([N_CORES, P, NMH, H2], CDT, addr_space="Shared")
            for mc in range(NM):
                ps = ps1.tile([P, H2], F32)
                nc.tensor.matmul(ps[:], lhsT=h1T[:, mc * P:(mc + 1) * P],
                                 rhs=w2t[:])
                ev = evict.tile([P, H2], CDT, tag="ev2")
                nc.vector.tensor_copy(ev[:], ps[:])
                dst = z2locA if mc < NMH else z2locB
                nc.sync.dma_start(dst[:, mc % NMH, :], ev[:])
            nc.gpsimd.collective_compute(
                "AllGather", mybir.AluOpType.bypass,
                replica_groups=[list(range(N_CORES))],
                ins=[z2locA[:].opt()], outs=[z2fullA[:].opt()],
            )
            nc.gpsimd.collective_compute(
                "AllGather", mybir.AluOpType.bypass,
                replica_groups=[list(range(N_CORES))],
                ins=[z2locB[:].opt()], outs=[z2fullB[:].opt()],
            )
            # gpsimd queue: keeps the sync queue free to prefetch the next
            # pass's adj stream while the collective completes
            Z2 = acts.tile([P, NK, H2], CDT)
            for j in range(N_CORES):
                nc.gpsimd.dma_start(Z2[:, j * NM:j * NM + NMH, :],
                                    z2fullA[j, :, :, :])
            for j in range(N_CORES):
                nc.gpsimd.dma_start(Z2[:, j * NM + NMH:(j + 1) * NM, :],
                                    z2fullB[j, :, :, :])
            # k-subtiles covered by the first half-gather, then the rest
            d_order = [j * NM + m for j in range(N_CORES) for m in range(NMH)]
            d_order += [j * NM + NMH + m for j in range(N_CORES)
                        for m in range(NMH)]

            # ---- phase D: h2T = relu(Z2.T @ adjT + b2) ----
            psD = [psacc.tile([H2, RC], F32, name=f"psD{i}", tag="accbank")
                   for i in range(NRC)]
            for ki, k in enumerate(d_order):
                at = stream.tile([P, RL], CDT, tag="adjstream")
                nc.sync.dma_start(at[:], adjt[k * P:(k + 1) * P, :])
                if pool_mode and k >= NK // 2:
                    nc.vector.reduce_sum(
                        PT[:, k, :],
                        at[:].rearrange("p (g s) -> p g s", s=SEG),
                        axis=mybir.AxisListType.X,
                    )
                for rc in range(NRC):
                    nc.tensor.matmul(
                        psD[rc][:], lhsT=Z2[:, k, :],
                        rhs=at[:, rc * RC:(rc + 1) * RC],
                        start=(ki == 0), stop=(ki == NK - 1),
                    )
            for rc in range(NRC):
                sl = slice(rc * RC, (rc + 1) * RC)
                nc.vector.tensor_scalar(
                    h2T[:, sl], psD[rc][:], INV_SCALE, b2t[:],
                    mybir.AluOpType.mult, mybir.AluOpType.add,
                )
                nc.vector.tensor_scalar(
                    h2T[:, sl], h2T[:, sl], 0.0, None, mybir.AluOpType.max,
                )

            # ---- phase E: Z3 local rows + AllGather ----
            ZDT = F32 if pool_mode else CDT  # pool matmul runs f32 (PT is f32)
            z3locA = dram.tile([P, NMH, L], ZDT)
            z3locB = dram.tile([P, NMH, L], ZDT)
            z3fullA = dram.tile([N_CORES, P, NMH, L], ZDT, addr_space="Shared")
            z3fullB = dram.tile([N_CORES, P, NMH, L], ZDT, addr_space="Shared")
            for mc in range(NM):
                ps = ps1.tile([P, L], F32)
                nc.tensor.matmul(ps[:], lhsT=h2T[:, mc * P:(mc + 1) * P],
                                 rhs=w3t[:])
                ev = evict.tile([P, L], ZDT, tag="ev3")
                nc.vector.tensor_copy(ev[:], ps[:])
                dst = z3locA if mc < NMH else z3locB
                nc.sync.dma_start(dst[:, mc % NMH, :], ev[:])
            nc.gpsimd.collective_compute(
                "AllGather", mybir.AluOpType.bypass,
                replica_groups=[list(range(N_CORES))],
                ins=[z3locA[:].opt()], outs=[z3fullA[:].opt()],
            )
            nc.gpsimd.collective_compute(
                "AllGather", mybir.AluOpType.bypass,
                replica_groups=[list(range(N_CORES))],
                ins=[z3locB[:].opt()], outs=[z3fullB[:].opt()],
            )
            Z3 = acts.tile([P, NK, L], ZDT)
            # in pool mode the adj stream is done, so the sync HWDGE queue is
            # idle and faster for this load than gpsimd SWDGE
            z3_dma = nc.sync.dma_start if pool_mode else nc.gpsimd.dma_start
            for j in range(N_CORES):
                z3_dma(Z3[:, j * NM:j * NM + NMH, :], z3fullA[j, :, :, :])
            for j in range(N_CORES):
                z3_dma(Z3[:, j * NM + NMH:(j + 1) * NM, :], z3fullB[j, :, :, :])

            if pool_mode:
                # ---- pooled output: out[g] = (Z3.T @ PT)[:, g] / SEG + b3 ----
                psP = psacc.tile([L, GL], F32, tag="accbank")
                for ki, k in enumerate(d_order):
                    nc.tensor.matmul(
                        psP[:], lhsT=Z3[:, k, :], rhs=PT[:, k, :],
                        start=(ki == 0), stop=(ki == NK - 1),
                    )
                poolT = acts.tile([L, GL], F32)
                nc.scalar.activation(
                    poolT[:], psP[:], AF.Identity,
                    bias=b3t[:], scale=INV_SCALE / SEG,
                )
                nc.sync.dma_start(poolt[:], poolT[:])
            else:
                # ---- phase F: encT = Z3.T @ adjT + b3 ----
                psF = [psacc.tile([L, RC], F32, name=f"psF{i}", tag="accbank")
                       for i in range(NRC)]
                for ki, k in enumerate(d_order):
                    at = stream.tile([P, RL], CDT, tag="adjstream")
                    nc.sync.dma_start(at[:], adjt[k * P:(k + 1) * P, :])
                    for rc in range(NRC):
                        nc.tensor.matmul(
                            psF[rc][:], lhsT=Z3[:, k, :],
                            rhs=at[:, rc * RC:(rc + 1) * RC],
                            start=(ki == 0), stop=(ki == NK - 1),
                        )
                for rc in range(NRC):
                    nc.scalar.activation(
                        encT[:, rc * RC:(rc + 1) * RC], psF[rc][:],
                        AF.Identity, bias=b3t[:], scale=INV_SCALE,
                    )
                nc.sync.dma_start(enct[:], encT[:])

    nc.compile()
    _BUILD_CACHE[key] = nc
    return nc


def _np32(a):
    a = np.asarray(a)
    if a.dtype == np.float32:
        return np.ascontiguousarray(a)
    return np.ascontiguousarray(a, dtype=np.float32)


def kernel(x, adj, graph_to_last_batch, w1, b1, w2, b2, w3, b3,
           ew1, eb1, ew2, eb2):
    global LAST_RESULT
    x = _np32(x)
    adj = _np32(adj)
    boundaries = np.asarray(graph_to_last_batch)
    w1 = _np32(w1); b1 = _np32(b1)
    w2 = _np32(w2); b2 = _np32(b2)
    w3 = _np32(w3); b3 = _np32(b3)

    clean = np.array_equal(
        boundaries.astype(np.int64),
        np.arange(1, G + 1, dtype=np.int64) * SEG)
    variant = "pool" if clean else "general"
    nc = _build(variant)
    cnp = _cnp()

    xt = np.ascontiguousarray(x.T).astype(cnp)
    shared = {
        "xt": xt,
        "w1": w1.astype(cnp), "b1": b1.reshape(H1, 1),
        "w2": w2.astype(cnp), "b2": b2.reshape(H2, 1),
        "w3": w3.astype(cnp), "b3": b3.reshape(L, 1),
    }
    in_maps = []
    for i in range(N_CORES):
        adjt_i = np.ascontiguousarray(adj[i * RL:(i + 1) * RL, :].T)
        if ADJ_SCALE != 1.0:
            adjt_i = adjt_i * np.float32(ADJ_SCALE)
        adjt_i = adjt_i.astype(cnp)
        in_maps.append({"adjt": adjt_i, **shared})

    res = run_bass_kernel_spmd(nc, in_maps, core_ids=list(range(N_CORES)))
    LAST_RESULT = res

    if clean:
        out = np.concatenate(
            [res.results[i]["poolt"].T for i in range(N_CORES)],
            axis=0).astype(np.float32)  # [G, L]
    else:
        encT = np.concatenate(
            [res.results[i]["enct"] for i in range(N_CORES)], axis=1)
        enc = encT.T.astype(np.float32)  # [N, L]
        # exact jax segment-mean semantics
        seg = np.searchsorted(boundaries, np.arange(N), side="right")
        valid = seg < G
        sums = np.zeros((G, L), np.float32)
        np.add.at(sums, seg[valid], enc[valid])
        cnts = np.zeros((G,), np.float32)
        np.add.at(cnts, seg[valid], np.float32(1.0))
        with np.errstate(divide="ignore", invalid="ignore"):
            out = (sums / cnts[:, None]).astype(np.float32)

    ew1 = _np32(ew1); eb1 = _np32(eb1)
    ew2 = _np32(ew2); eb2 = _np32(eb2)
    e = np.maximum(out @ ew1 + eb1, 0.0).astype(np.float32)
    logits = (e @ ew2 + eb2).astype(np.float32)
    zm = logits - logits.max(axis=1, keepdims=True)
    ez = np.exp(zm)
    gamma = (ez / ez.sum(axis=1, keepdims=True)).astype(np.float32)

    return (x, out, gamma)
